# revision 9
# baseline (speedup 1.0000x reference)
"""Multi-head attention (B=2, T=2048, C=1024, H=16, hd=64, RoPE, full mask)
on 8 TRN2 NeuronCores.

Sharding: tensor-parallel over (batch, head-group). Core c handles batch
c//4 and heads [4*(c%4) .. 4*(c%4)+3]. Each core computes the QKV
projection for its 4 heads, full attention over T=2048, and a partial
output projection y = o_heads @ w_proj[:, cols].T. The host sums the 4
partial y's per batch (the tensor-parallel unshard reduction).

On-chip layout is "transposed everything" so attention needs no on-chip
transposes of the big tensors:
  - qT/kT stored [head_dim, T] (projection computed as w @ x.T)
  - scores computed directly transposed: sT[tk, tq] = k[tk] . q[tq]
  - softmax denominator via an appended ones-column on V (M=66 matmul)
  - o.T scaled by 1/den via per-pair gather + batched reciprocal +
    0/1-matrix broadcast matmul
RoPE uses full-width elementwise ops after a host-side even/odd row
permutation of w_q/w_k plus a DVE stream_shuffle that swaps adjacent
32-partition blocks.

Schedule: the scalar-engine exp over the T^2 scores (~142us/core) is the
binding resource. The program is ONE globally software-pipelined tick
stream over all 128 (qb, pair, kb) ticks with the scores matmul running
one tick AHEAD of the exp and AV one tick behind:
    tick j:  [pump fillers] scores(j) | exp(j-1) | AV(j-2)
so filler work pumped between ticks can never stall the exp stream.
Everything outside the prologue (k(n0), q(qb0)) is a filler unit with a
deadline: remaining qkv projections, RoPE, per-pair finalize chains, and
output-projection chunks. Priority-ordered slab DMAs + PE/ACT warmup
shrink the head; the per-pair finalize split shrinks the tail.

Precision: f16 operands with fp32 PSUM accumulation everywhere.
"""

import heapq

import ml_dtypes  # noqa: F401
import numpy as np

import concourse.bacc as bacc
import concourse.mybir as mybir
import concourse.tile as tile
from concourse.bass_utils import run_bass_kernel_spmd

# Problem constants (hardcoded per contract)
B, T, C = 2, 2048, 1024
N_HEAD = 16
HD = 64
N_CORES = 8
HPC = 4  # heads per core
GC = HPC * HD  # head channels per core = 256

P = 128
KC = C // P  # 8 contraction chunks for the projections
NQB = 4  # query blocks
TQ = T // NQB  # 512
NKB = T // P  # 16 key blocks
VW = HD + 2  # 66: v + ones col + pad col
NT = 4 * 32  # total ticks

F32 = mybir.dt.float32
F32R = mybir.dt.float32r
F16 = mybir.dt.float16

_PROGRAM = None


def _build_program():
    nc = bacc.Bacc(
        "TRN2", target_bir_lowering=False, debug=False, num_devices=N_CORES
    )

    xT_d = nc.dram_tensor("xT", [C, T], F16, kind="ExternalInput").ap()
    wqkT_d = nc.dram_tensor("wqkT", [C, 4 * P], F16, kind="ExternalInput").ap()
    wvT_d = nc.dram_tensor("wvT", [C, GC], F16, kind="ExternalInput").ap()
    wpT_d = nc.dram_tensor("wpT", [GC, C], F16, kind="ExternalInput").ap()
    cc_d = nc.dram_tensor("cc", [P, T], F16, kind="ExternalInput").ap()
    ss_d = nc.dram_tensor("ss", [P, T], F16, kind="ExternalInput").ap()
    emat_d = nc.dram_tensor("emat", [2, P], F32R, kind="ExternalInput").ap()
    ident_d = nc.dram_tensor("ident", [P, P], F32, kind="ExternalInput").ap()
    esel_d = nc.dram_tensor("esel", [P, HPC], F32, kind="ExternalInput").ap()
    y_d = nc.dram_tensor("y", [T, C], F32, kind="ExternalOutput").ap()

    # stream_shuffle permutes WITHIN each 32-partition block (mask is per
    # partition, replicated across blocks). The host lays q/k head rows as
    # [even0:16, odd0:16, even16:32, odd16:32] so the RoPE partner sits at
    # p^16 inside the block.
    SHUF_MASK = [i ^ 16 for i in range(32)]

    with tile.TileContext(nc) as tc:
        with (
            tc.tile_pool(name="consts", bufs=1) as consts,
            tc.tile_pool(name="bigs", bufs=1) as bigs,
            tc.tile_pool(name="tmps", bufs=2) as tmps,
            tc.tile_pool(name="expool", bufs=3) as expool,
            tc.tile_pool(name="psS", bufs=2, space="PSUM") as psS,
            tc.tile_pool(name="psW", bufs=2, space="PSUM") as psW,
            tc.tile_pool(name="psO", bufs=2, space="PSUM") as psO,
        ):
            # ---- resident tiles ----
            x_big = bigs.tile([P, KC * T], F16, tag="xbig", name="xbig")
            x3 = x_big.rearrange("p (kc t) -> p kc t", t=T)
            wqk_big = bigs.tile([P, KC * 4 * P], F16, tag="wqkbig", name="wqkbig")
            wqk3 = wqk_big.rearrange("p (kc m) -> p kc m", m=4 * P)
            wv_big = bigs.tile([P, KC * GC], F16, tag="wvbig", name="wvbig")
            wv3 = wv_big.rearrange("p (kc m) -> p kc m", m=GC)
            wp_big = bigs.tile([P, 2 * C], F16, tag="wpbig", name="wpbig")
            wp3 = wp_big.rearrange("p (kb m) -> p kb m", m=C)
            cc_t = consts.tile([P, T], F16, tag="cc")
            ss_t = consts.tile([P, T], F16, tag="ss")
            emat_t = consts.tile([2, P], F32R, tag="emat")
            ident_t = consts.tile([P, P], F32, tag="ident")
            esel_t = consts.tile([P, HPC], F32, tag="esel")

            xsrc = xT_d.rearrange("(kc p) t -> p kc t", p=P)
            wqksrc = wqkT_d.rearrange("(kc p) m -> p kc m", p=P)
            wvsrc = wvT_d.rearrange("(kc p) m -> p kc m", p=P)
            wpsrc = wpT_d.rearrange("(kb p) m -> p kb m", p=P)

            # ---- warmup: ramp the PE p-state during the DMA wait and
            # preload the ACT exp table. exp(0*x)=1 makes the ones tile.
            warm = consts.tile([P, TQ], F16, tag="warm")
            nc.vector.memset(warm, 0.0)
            wps = psW.tile([P, TQ], F32, tag="aux", name="warmps")
            for i in range(4):
                nc.tensor.matmul(
                    wps, lhsT=warm[:, 0:P], rhs=warm, start=(i == 0), stop=(i == 3)
                )
            ones_f = consts.tile([P, TQ], F32, tag="ones_f")
            nc.scalar.activation(
                out=ones_f,
                in_=wps,
                func=mybir.ActivationFunctionType.Exp,
                scale=0.0,
            )
            ones4 = ones_f[:, 0 : 2 * HPC].rearrange("p (h c) -> p h c", c=2)

            # ---- DMAs in priority order (deps of early compute first) ----
            nc.sync.dma_start(
                out=wqk3[:, :, 2 * P : 4 * P], in_=wqksrc[:, :, 2 * P : 4 * P]
            )  # k weights
            nc.sync.dma_start(out=x3[:, :, 0:TQ], in_=xsrc[:, :, 0:TQ])  # x n0
            nc.sync.dma_start(out=cc_t[:, 0:TQ], in_=cc_d[:, 0:TQ])
            nc.sync.dma_start(out=ss_t[:, 0:TQ], in_=ss_d[:, 0:TQ])
            nc.sync.dma_start(
                out=wqk3[:, :, 0 : 2 * P], in_=wqksrc[:, :, 0 : 2 * P]
            )  # q weights
            nc.sync.dma_start(out=wv3, in_=wvsrc)
            nc.sync.dma_start(out=x3[:, :, TQ:T], in_=xsrc[:, :, TQ:T])  # x n1-3
            nc.sync.dma_start(out=cc_t[:, TQ:T], in_=cc_d[:, TQ:T])
            nc.sync.dma_start(out=ss_t[:, TQ:T], in_=ss_d[:, TQ:T])
            nc.sync.dma_start(out=wp3, in_=wpsrc)
            nc.sync.dma_start(out=emat_t, in_=emat_d)
            nc.sync.dma_start(out=ident_t, in_=ident_d)
            nc.sync.dma_start(out=esel_t, in_=esel_d)

            qk_sb = [
                bigs.tile([P, T], F16, tag=f"qk{mb}", name=f"qk{mb}")
                for mb in range(4)
            ]
            va_list = [
                bigs.tile([P, HPC * VW], F16, tag=f"va{tb}", name=f"va{tb}")
                for tb in range(NKB)
            ]

            # ---- work-unit emitters ----
            def proj_qk_tile(mb, n):
                """One qk projection tile + RoPE. Atomic unit: the PSUM
                tile must not live across filler units (a later psW alloc
                waiting on this unit's not-yet-emitted consumer would
                deadlock the in-order PE queue)."""
                ns = slice(n * TQ, (n + 1) * TQ)
                ps = psW.tile([P, TQ], F32, tag="aux", name=f"ps{mb}_{n}")
                for kc in range(KC):
                    nc.tensor.matmul(
                        ps,
                        lhsT=wqk_big[
                            :, kc * 4 * P + mb * P : kc * 4 * P + (mb + 1) * P
                        ],
                        rhs=x_big[:, kc * T + n * TQ : kc * T + (n + 1) * TQ],
                        start=(kc == 0),
                        stop=(kc == KC - 1),
                    )
                sb = qk_sb[mb]
                nc.vector.tensor_copy(sb[:, ns], ps)
                # RoPE: even/odd lanes premixed in the weights; the cross
                # term needs partitions p <-> p^32, done on DVE.
                shuf = tmps.tile([P, TQ], F16, tag="shuf")
                nc.vector.stream_shuffle(shuf, sb[:, ns], SHUF_MASK)
                nc.vector.tensor_mul(sb[:, ns], sb[:, ns], cc_t[:, ns])
                tmp = tmps.tile([P, TQ], F16, tag="ropetmp")
                nc.vector.tensor_mul(tmp, shuf, ss_t[:, ns])
                nc.vector.tensor_add(sb[:, ns], sb[:, ns], tmp)

            def proj_v_tb(tb):
                vp = psW.tile([P, TQ], F32, tag="aux", name=f"vp{tb}")
                vps = vp[:, 0:GC]
                for kc in range(KC):
                    nc.tensor.matmul(
                        vps,
                        lhsT=x_big[:, kc * T + tb * P : kc * T + (tb + 1) * P],
                        rhs=wv_big[:, kc * GC : (kc + 1) * GC],
                        start=(kc == 0),
                        stop=(kc == KC - 1),
                    )
                va = va_list[tb]
                va4 = va.rearrange("p (h c) -> p h c", c=VW)
                nc.vector.tensor_copy(va4[:, :, HD : HD + 2], ones4)
                nc.vector.tensor_copy(
                    va4[:, :, 0:HD], vps.rearrange("p (h c) -> p h c", c=HD)
                )

            # ---- deadline-scheduled filler pump ----
            fillers = []  # heap of (deadline, seq, cost, fn)
            fseq = [0]
            credit = [0.0]
            ACT_NS = 1150.0
            BASE_NS = 820.0

            def add_filler(deadline, cost, fn):
                heapq.heappush(fillers, (deadline, fseq[0], cost, fn))
                fseq[0] += 1

            def pump(j):
                credit[0] = min(credit[0] + (ACT_NS - BASE_NS), 2400.0)
                while fillers and fillers[0][0] <= j:
                    _, _, c, fn = heapq.heappop(fillers)
                    fn()
                    credit[0] -= c
                credit[0] = max(credit[0], -1500.0)
                while fillers and credit[0] >= fillers[0][2]:
                    _, _, c, fn = heapq.heappop(fillers)
                    fn()
                    credit[0] -= c

            # ---- pipelined attention stages over global ticks ----
            # tick g = qb*32 + p*16 + kb
            st2_live = {}
            ex_live = {}
            oau_live = {}
            qdat = {}  # qb -> (oevp, den4)
            odat = {}  # qb -> [o_sb0, o_sb1]

            def scores_stage(g):
                qb, r = divmod(g, 32)
                p, kb = divmod(r, 16)
                qs = slice(qb * TQ, (qb + 1) * TQ)
                ks = slice(kb * P, (kb + 1) * P)
                qt = qk_sb[p]
                kt = qk_sb[2 + p]
                st2 = psS.tile([P, 2 * TQ], F32, tag="st2", name=f"st2_{g}")
                for i in range(2):
                    nc.tensor.matmul(
                        st2[:, i * TQ : (i + 1) * TQ],
                        lhsT=kt[i * HD : (i + 1) * HD, ks],
                        rhs=qt[i * HD : (i + 1) * HD, qs],
                        start=True,
                        stop=True,
                    )
                st2_live[g] = st2

            def exp_stage(g):
                st2 = st2_live.pop(g)
                ex = expool.tile([P, 2 * TQ], F16, tag="ex", name=f"ex_{g}")
                nc.scalar.activation(
                    out=ex,
                    in_=st2,
                    func=mybir.ActivationFunctionType.Exp,
                    scale=1.0 / np.sqrt(HD),
                )
                ex_live[g] = ex

            def av_stage(g):
                qb, r = divmod(g, 32)
                p, kb = divmod(r, 16)
                if p == 0 and kb == 0:
                    oevp = [
                        tmps.tile(
                            [P, TQ], F32, tag=f"oevp{pp}",
                            name=f"oevp{pp}_{qb}", bufs=2,
                        )
                        for pp in range(2)
                    ]
                    den4 = tmps.tile(
                        [P, TQ], F32, tag="den4", name=f"den4_{qb}", bufs=2
                    )
                    nc.vector.memset(den4, 1.0)
                    qdat[qb] = (oevp, den4)
                if kb == 0:
                    oau_live[(qb, p)] = [
                        psO.tile([VW, TQ], F32, tag="oau", name=f"oau{i}_{qb}{p}")
                        for i in range(2)
                    ]
                oau = oau_live[(qb, p)]
                ex = ex_live.pop(g)
                for i in range(2):
                    h = 2 * p + i
                    nc.tensor.matmul(
                        oau[i],
                        lhsT=va_list[kb][:, h * VW : h * VW + VW],
                        rhs=ex[:, i * TQ : (i + 1) * TQ],
                        start=(kb == 0),
                        stop=(kb == NKB - 1),
                    )
                if kb == NKB - 1:
                    oevp, den4 = qdat[qb]
                    for i in range(2):
                        nc.vector.tensor_copy(
                            oevp[p][i * HD : (i + 1) * HD, :], oau[i][0:HD, :]
                        )
                        r0 = 32 * (2 * p + i)
                        nc.vector.tensor_copy(
                            den4[r0 : r0 + 1, :], oau[i][HD : HD + 1, :]
                        )
                    del oau_live[(qb, p)]
                    if p == 0:
                        add_filler(qb * 32 + 22, 900.0, lambda q=qb: fin_pair(q, 0))
                    else:
                        add_filler(qb * 32 + 36, 900.0, lambda q=qb: fin_pair(q, 1))
                        for tch in range(TQ // P):
                            add_filler(
                                qb * 32 + 38 + 4 * tch,
                                1200.0,
                                lambda q=qb, t=tch: yproj_tch(q, t),
                            )

            # ---- finalize: per-pair reciprocal chain, then y projection ----
            def fin_pair(qb, p):
                oevp, den4 = qdat[qb]
                o_sb = tmps.tile(
                    [P, TQ], F16, tag=f"osb{p}", name=f"osb{p}_{qb}", bufs=2
                )
                odat.setdefault(qb, [None, None])[p] = o_sb
                denT = psW.tile([P, 2 * HPC], F32, tag="aux", name=f"denT{qb}{p}")
                for c in range(4):
                    nc.tensor.matmul(
                        denT[:, c * 2 : (c + 1) * 2],
                        lhsT=den4[:, c * P : (c + 1) * P],
                        rhs=esel_t[:, 2 * p : 2 * p + 2],
                        start=True,
                        stop=True,
                    )
                rdenT = tmps.tile([P, 2 * HPC], F32, tag="rdenT")
                nc.vector.reciprocal(rdenT, denT)
                rden_ps = psW.tile([2, TQ], F32, tag="aux", name=f"rdps{qb}{p}")
                for c in range(4):
                    nc.tensor.transpose(
                        rden_ps[:, c * P : (c + 1) * P],
                        rdenT[:, c * 2 : (c + 1) * 2],
                        ident_t,
                    )
                rden2 = tmps.tile([2, TQ], F32R, tag="rden2")
                with nc.allow_low_precision(reason="f32r round of 1/den"):
                    nc.vector.tensor_copy(rden2, rden_ps)
                bc = psW.tile([P, TQ], F32, tag="aux", name=f"bc{qb}{p}")
                nc.tensor.matmul(bc, lhsT=emat_t, rhs=rden2, start=True, stop=True)
                nc.vector.tensor_mul(o_sb, oevp[p], bc)
                if p == 1:
                    del qdat[qb]

            def yproj_tch(qb, tch):
                o_sb = odat[qb]
                for cch in range(C // TQ):
                    yp = psW.tile([P, TQ], F32, tag="aux", name="yp")
                    for kb in range(2):
                        nc.tensor.matmul(
                            yp,
                            lhsT=o_sb[kb][:, tch * P : (tch + 1) * P],
                            rhs=wp_big[
                                :, kb * C + cch * TQ : kb * C + (cch + 1) * TQ
                            ],
                            start=(kb == 0),
                            stop=(kb == 1),
                        )
                    ysb = tmps.tile([P, TQ], F32, tag="ysb")
                    nc.vector.tensor_copy(ysb, yp)
                    r0 = qb * TQ + tch * P
                    nc.sync.dma_start(
                        out=y_d[r0 : r0 + P, cch * TQ : (cch + 1) * TQ],
                        in_=ysb,
                    )
                if tch == TQ // P - 1:
                    del odat[qb]

            # ---- prologue: k(pair0, n0) and q(pair0, n0) directly ----
            proj_qk_tile(2, 0)
            proj_qk_tile(0, 0)

            # ---- seed filler units with deadlines (j-space) ----
            # v(tb): consumed by AV(qb0, p0, kb=tb) at j = tb+2.
            for tb in range(NKB):
                add_filler(max(tb, 1), 1250.0, lambda t=tb: proj_v_tb(t))
            # k tiles: pair0 scores(kb=4nb) at j=4nb; pair1 at j=16+4nb.
            for nb in range(1, NQB):
                add_filler(4 * nb - 1, 2100.0, lambda n=nb: proj_qk_tile(2, n))
            add_filler(13, 2100.0, lambda: proj_qk_tile(3, 0))
            for nb in range(1, NQB):
                add_filler(16 + 4 * nb - 1, 2100.0, lambda n=nb: proj_qk_tile(3, n))
            # q tiles: qb=nb pair0 at j=32nb; pair1 at j=32nb+16.
            add_filler(14, 2100.0, lambda: proj_qk_tile(1, 0))
            for nb in range(1, NQB):
                add_filler(32 * nb - 3, 2100.0, lambda n=nb: proj_qk_tile(0, n))
                add_filler(32 * nb + 13, 2100.0, lambda n=nb: proj_qk_tile(1, n))

            # ---- main pipelined loop ----
            for j in range(NT + 2):
                pump(j)
                if j < NT:
                    scores_stage(j)
                if 1 <= j <= NT:
                    exp_stage(j - 1)
                if 2 <= j:
                    av_stage(j - 2)
            while fillers:
                _, _, _, fn = heapq.heappop(fillers)
                fn()

    nc.compile()
    return nc


def _get_program():
    global _PROGRAM
    if _PROGRAM is None:
        _PROGRAM = _build_program()
    return _PROGRAM


def _eo(w):
    """[64, C] head rows -> [even0:16; odd0:16; even16:32; odd16:32] so the
    RoPE partner is at p^16 within a 32-partition block (stream_shuffle)."""
    e = w[0::2]
    o = w[1::2]
    return np.concatenate([e[0:16], o[0:16], e[16:32], o[16:32]], axis=0)


def _host_prep(x, cos, sin, w_qkv, w_proj):
    """Build the 8 per-core input maps."""
    f16 = np.float16
    xT = [np.ascontiguousarray(x[b].T).astype(f16) for b in range(B)]  # [C, T]

    cosT = np.ascontiguousarray(cos.T)  # [32, T]
    sinT = np.ascontiguousarray(sin.T)
    # per-head 64-row block matching _eo: rows [e0:16, o0:16, e16:32, o16:32]
    cc64 = np.concatenate([cosT[0:16], cosT[0:16], cosT[16:32], cosT[16:32]])
    ss64 = np.concatenate([-sinT[0:16], sinT[0:16], -sinT[16:32], sinT[16:32]])
    cc = np.tile(cc64, (2, 1)).astype(f16)  # [128, T]
    ss = np.tile(ss64, (2, 1)).astype(f16)
    emat = np.zeros((2, P), dtype=np.float32)
    for i in range(2):
        emat[i, i * HD : (i + 1) * HD] = 1.0
    ident = np.eye(P, dtype=np.float32)
    esel = np.zeros((P, HPC), dtype=np.float32)
    for j in range(HPC):
        esel[32 * j, j] = 1.0

    wq = w_qkv[0:C]
    wk = w_qkv[C : 2 * C]
    wv = w_qkv[2 * C : 3 * C]

    in_maps = []
    for core in range(N_CORES):
        b = core // 4
        h0 = 4 * (core % 4)
        heads = [h0, h0 + 1, h0 + 2, h0 + 3]
        blocks = []
        for pair in range(2):
            ha, hb = heads[2 * pair], heads[2 * pair + 1]
            blocks.append(
                np.concatenate(
                    [_eo(wq[ha * HD : ha * HD + HD]),
                     _eo(wq[hb * HD : hb * HD + HD])],
                    axis=0,
                )
            )
        for pair in range(2):
            ha, hb = heads[2 * pair], heads[2 * pair + 1]
            blocks.append(
                np.concatenate(
                    [_eo(wk[ha * HD : ha * HD + HD]),
                     _eo(wk[hb * HD : hb * HD + HD])],
                    axis=0,
                )
            )
        wqkT = np.ascontiguousarray(
            np.concatenate(blocks, axis=0).T
        ).astype(f16)  # [C, 512]
        wvT = np.ascontiguousarray(
            wv[h0 * HD : h0 * HD + GC].T
        ).astype(f16)  # [C, 256]
        wpT = np.ascontiguousarray(
            w_proj[:, h0 * HD : h0 * HD + GC].T
        ).astype(f16)  # [256, C]
        in_maps.append(
            {
                "xT": xT[b],
                "wqkT": wqkT,
                "wvT": wvT,
                "wpT": wpT,
                "cc": cc,
                "ss": ss,
                "emat": emat,
                "ident": ident,
                "esel": esel,
            }
        )
    return in_maps


def kernel(x, cos, sin, mask, w_qkv, w_proj, _trace=False, _tmpdir=None):
    x = np.asarray(x, dtype=np.float32)
    cos = np.asarray(cos, dtype=np.float32)
    sin = np.asarray(sin, dtype=np.float32)
    w_qkv = np.asarray(w_qkv, dtype=np.float32)
    w_proj = np.asarray(w_proj, dtype=np.float32)
    # mask is all-ones in this problem spec: no-op in the math.

    nc = _get_program()
    in_maps = _host_prep(x, cos, sin, w_qkv, w_proj)
    res = run_bass_kernel_spmd(
        nc, in_maps, list(range(N_CORES)), trace=_trace, tmpdir=_tmpdir
    )
    out = np.empty((B, T, C), dtype=np.float32)
    for b in range(B):
        acc = res.results[4 * b]["y"].astype(np.float32).copy()
        for g in range(1, 4):
            acc += res.results[4 * b + g]["y"]
        out[b] = acc
    kernel._last_exec_time_ns = res.exec_time_ns
    return out


# revision 15
# speedup vs baseline: 1.1923x; 1.1923x over previous
"""Multi-head attention (B=2, T=2048, C=1024, H=16, hd=64, RoPE, full mask)
on 8 TRN2 NeuronCores.

Sharding: tensor-parallel over (batch, head-group). Core c handles batch
c//4 and heads [4*(c%4) .. 4*(c%4)+3]. Each core computes the QKV
projection for its 4 heads, full attention over T=2048, and a partial
output projection y = o_heads @ w_proj[:, cols].T. The host sums the 4
partial y's per batch (the tensor-parallel unshard reduction).

On-chip layout is "transposed everything" so attention needs no on-chip
transposes of the big tensors:
  - qT/kT stored [head_dim, T] (projection computed as w @ x.T)
  - scores computed directly transposed: sT[tk, tq] = k[tk] . q[tq]
  - softmax denominator via an appended ones-column on V (M=66 matmul)
  - o.T scaled by 1/den via per-pair gather + batched reciprocal +
    0/1-matrix broadcast matmul
RoPE uses full-width elementwise ops after a host-side even/odd row
permutation of w_q/w_k plus a DVE stream_shuffle that swaps adjacent
32-partition blocks.

Schedule: the scalar-engine exp over the T^2 scores (~142us/core) is the
binding resource. The program is ONE globally software-pipelined tick
stream over all 128 (qb, pair, kb) ticks with the scores matmul running
one tick AHEAD of the exp and AV one tick behind:
    tick j:  [pump fillers] scores(j) | exp(j-1) | AV(j-2)
so filler work pumped between ticks can never stall the exp stream.
Everything outside the prologue (k(n0), q(qb0)) is a filler unit with a
deadline: remaining qkv projections, RoPE, per-pair finalize chains, and
output-projection chunks. Priority-ordered slab DMAs + PE/ACT warmup
shrink the head; the per-pair finalize split shrinks the tail.

Precision: f16 operands with fp32 PSUM accumulation everywhere.
"""

import heapq

import ml_dtypes  # noqa: F401
import numpy as np

import concourse.bacc as bacc
import concourse.mybir as mybir
import concourse.tile as tile
from concourse.bass_utils import run_bass_kernel_spmd

# Problem constants (hardcoded per contract)
B, T, C = 2, 2048, 1024
N_HEAD = 16
HD = 64
N_CORES = 8
HPC = 4  # heads per core
GC = HPC * HD  # head channels per core = 256

P = 128
KC = C // P  # 8 contraction chunks for the projections
NQB = 4  # query blocks
TQ = T // NQB  # 512
NKB = T // P  # 16 key blocks
VW = HD + 2  # 66: v + ones col + pad col
NT = 4 * 32  # total ticks

F32 = mybir.dt.float32
F32R = mybir.dt.float32r
F16 = mybir.dt.float16

_PROGRAM = None


def _build_program():
    nc = bacc.Bacc(
        "TRN2", target_bir_lowering=False, debug=False, num_devices=N_CORES
    )

    xT_d = nc.dram_tensor("xT", [C, T], F16, kind="ExternalInput").ap()
    wqkT_d = nc.dram_tensor("wqkT", [C, 4 * P], F16, kind="ExternalInput").ap()
    wvT_d = nc.dram_tensor("wvT", [C, GC], F16, kind="ExternalInput").ap()
    wpT_d = nc.dram_tensor("wpT", [GC, C], F16, kind="ExternalInput").ap()
    cc_d = nc.dram_tensor("cc", [P, T], F16, kind="ExternalInput").ap()
    ss_d = nc.dram_tensor("ss", [P, T], F16, kind="ExternalInput").ap()
    emat_d = nc.dram_tensor("emat", [2, P], F32R, kind="ExternalInput").ap()
    ident_d = nc.dram_tensor("ident", [P, P], F32, kind="ExternalInput").ap()
    esel_d = nc.dram_tensor("esel", [P, HPC], F32, kind="ExternalInput").ap()
    y_d = nc.dram_tensor("y", [T, C], F32, kind="ExternalOutput").ap()

    # stream_shuffle permutes WITHIN each 32-partition block (mask is per
    # partition, replicated across blocks). The host lays q/k head rows as
    # [even0:16, odd0:16, even16:32, odd16:32] so the RoPE partner sits at
    # p^16 inside the block.
    SHUF_MASK = [i ^ 16 for i in range(32)]

    with tile.TileContext(nc) as tc:
        with (
            tc.tile_pool(name="consts", bufs=1) as consts,
            tc.tile_pool(name="bigs", bufs=1) as bigs,
            tc.tile_pool(name="tmps", bufs=2) as tmps,
            tc.tile_pool(name="expool", bufs=4) as expool,
            tc.tile_pool(name="psS", bufs=2, space="PSUM") as psS,
            tc.tile_pool(name="psW", bufs=2, space="PSUM") as psW,
            tc.tile_pool(name="psO", bufs=2, space="PSUM") as psO,
        ):
            # ---- resident tiles ----
            x_big = bigs.tile([P, KC * T], F16, tag="xbig", name="xbig")
            x3 = x_big.rearrange("p (kc t) -> p kc t", t=T)
            wqk_big = bigs.tile([P, KC * 4 * P], F16, tag="wqkbig", name="wqkbig")
            wqk3 = wqk_big.rearrange("p (kc m) -> p kc m", m=4 * P)
            wv_big = bigs.tile([P, KC * GC], F16, tag="wvbig", name="wvbig")
            wv3 = wv_big.rearrange("p (kc m) -> p kc m", m=GC)
            wp_big = bigs.tile([P, 2 * C], F16, tag="wpbig", name="wpbig")
            wp3 = wp_big.rearrange("p (kb m) -> p kb m", m=C)
            cc_t = consts.tile([P, T], F16, tag="cc")
            ss_t = consts.tile([P, T], F16, tag="ss")
            emat_t = consts.tile([2, P], F32R, tag="emat")
            ident_t = consts.tile([P, P], F32, tag="ident")
            esel_t = consts.tile([P, HPC], F32, tag="esel")

            xsrc = xT_d.rearrange("(kc p) t -> p kc t", p=P)
            wqksrc = wqkT_d.rearrange("(kc p) m -> p kc m", p=P)
            wvsrc = wvT_d.rearrange("(kc p) m -> p kc m", p=P)
            wpsrc = wpT_d.rearrange("(kb p) m -> p kb m", p=P)

            # ---- warmup: ramp the PE p-state during the DMA wait and
            # preload the ACT exp table. exp(0*x)=1 makes the ones tile.
            warm = consts.tile([P, TQ], F16, tag="warm")
            nc.vector.memset(warm, 0.0)
            wps = psW.tile([P, TQ], F32, tag="aux", name="warmps")
            for i in range(4):
                nc.tensor.matmul(
                    wps, lhsT=warm[:, 0:P], rhs=warm, start=(i == 0), stop=(i == 3)
                )
            ones_f = consts.tile([P, TQ], F32, tag="ones_f")
            nc.scalar.activation(
                out=ones_f,
                in_=wps,
                func=mybir.ActivationFunctionType.Exp,
                scale=0.0,
            )
            ones4 = ones_f[:, 0 : 2 * HPC].rearrange("p (h c) -> p h c", c=2)

            # ---- DMAs in priority order (deps of early compute first) ----
            nc.sync.dma_start(
                out=wqk3[:, :, 2 * P : 4 * P], in_=wqksrc[:, :, 2 * P : 4 * P]
            )  # k weights
            nc.sync.dma_start(out=x3[:, :, 0:TQ], in_=xsrc[:, :, 0:TQ])  # x n0
            nc.sync.dma_start(out=cc_t[:, 0:TQ], in_=cc_d[:, 0:TQ])
            nc.sync.dma_start(out=ss_t[:, 0:TQ], in_=ss_d[:, 0:TQ])
            nc.sync.dma_start(
                out=wqk3[:, :, 0 : 2 * P], in_=wqksrc[:, :, 0 : 2 * P]
            )  # q weights
            nc.sync.dma_start(out=wv3, in_=wvsrc)
            # x n1..n3 as separate slabs so each lands as early as possible
            # (a deadline-pumped v-projection blocking the in-order PE queue
            # on a late x DMA would starve the exp stream behind it)
            for nb in range(1, NQB):
                nc.sync.dma_start(
                    out=x3[:, :, nb * TQ : (nb + 1) * TQ],
                    in_=xsrc[:, :, nb * TQ : (nb + 1) * TQ],
                )
            nc.sync.dma_start(out=cc_t[:, TQ:T], in_=cc_d[:, TQ:T])
            nc.sync.dma_start(out=ss_t[:, TQ:T], in_=ss_d[:, TQ:T])
            nc.sync.dma_start(out=wp3, in_=wpsrc)
            nc.sync.dma_start(out=emat_t, in_=emat_d)
            nc.sync.dma_start(out=ident_t, in_=ident_d)
            nc.sync.dma_start(out=esel_t, in_=esel_d)

            qk_sb = [
                bigs.tile([P, T], F16, tag=f"qk{mb}", name=f"qk{mb}")
                for mb in range(4)
            ]
            va_list = [
                bigs.tile([P, HPC * VW], F16, tag=f"va{tb}", name=f"va{tb}")
                for tb in range(NKB)
            ]

            # ---- work-unit emitters ----
            def proj_qk_tile(mb, n):
                """One qk projection tile + RoPE. Atomic unit: the PSUM
                tile must not live across filler units (a later psW alloc
                waiting on this unit's not-yet-emitted consumer would
                deadlock the in-order PE queue)."""
                ns = slice(n * TQ, (n + 1) * TQ)
                ps = psW.tile([P, TQ], F32, tag="aux", name=f"ps{mb}_{n}")
                for kc in range(KC):
                    nc.tensor.matmul(
                        ps,
                        lhsT=wqk_big[
                            :, kc * 4 * P + mb * P : kc * 4 * P + (mb + 1) * P
                        ],
                        rhs=x_big[:, kc * T + n * TQ : kc * T + (n + 1) * TQ],
                        start=(kc == 0),
                        stop=(kc == KC - 1),
                    )
                sb = qk_sb[mb]
                nc.vector.tensor_copy(sb[:, ns], ps)
                # RoPE: even/odd lanes premixed in the weights; the cross
                # term needs partitions p <-> p^32, done on DVE.
                shuf = tmps.tile([P, TQ], F16, tag="shuf")
                nc.vector.stream_shuffle(shuf, sb[:, ns], SHUF_MASK)
                nc.vector.tensor_mul(sb[:, ns], sb[:, ns], cc_t[:, ns])
                tmp = tmps.tile([P, TQ], F16, tag="ropetmp")
                nc.vector.tensor_mul(tmp, shuf, ss_t[:, ns])
                nc.vector.tensor_add(sb[:, ns], sb[:, ns], tmp)

            def proj_v_tb(tb):
                vp = psW.tile([P, TQ], F32, tag="aux", name=f"vp{tb}")
                vps = vp[:, 0:GC]
                for kc in range(KC):
                    nc.tensor.matmul(
                        vps,
                        lhsT=x_big[:, kc * T + tb * P : kc * T + (tb + 1) * P],
                        rhs=wv_big[:, kc * GC : (kc + 1) * GC],
                        start=(kc == 0),
                        stop=(kc == KC - 1),
                    )
                va = va_list[tb]
                va4 = va.rearrange("p (h c) -> p h c", c=VW)
                nc.vector.tensor_copy(va4[:, :, HD : HD + 2], ones4)
                nc.vector.tensor_copy(
                    va4[:, :, 0:HD], vps.rearrange("p (h c) -> p h c", c=HD)
                )

            # ---- deadline-scheduled filler pump ----
            fillers = []  # heap of (deadline, seq, cost, fn)
            fseq = [0]
            credit = [0.0]
            ACT_NS = 1150.0
            BASE_NS = 820.0

            def add_filler(deadline, cost, fn):
                heapq.heappush(fillers, (deadline, fseq[0], cost, fn))
                fseq[0] += 1

            def pump(j):
                credit[0] = min(credit[0] + (ACT_NS - BASE_NS), 2400.0)
                while fillers and fillers[0][0] <= j:
                    _, _, c, fn = heapq.heappop(fillers)
                    fn()
                    credit[0] -= c
                credit[0] = max(credit[0], -1500.0)
                while fillers and credit[0] >= fillers[0][2]:
                    _, _, c, fn = heapq.heappop(fillers)
                    fn()
                    credit[0] -= c

            # ---- pipelined attention stages over global ticks ----
            # tick g = qb*32 + p*16 + kb
            st2_live = {}
            ex_live = {}
            oau_live = {}
            qdat = {}  # qb -> (oevp, den4)
            odat = {}  # qb -> [o_sb0, o_sb1]

            def scores_stage(g):
                qb, r = divmod(g, 32)
                p, kb = divmod(r, 16)
                qs = slice(qb * TQ, (qb + 1) * TQ)
                ks = slice(kb * P, (kb + 1) * P)
                qt = qk_sb[p]
                kt = qk_sb[2 + p]
                st2 = psS.tile([P, 2 * TQ], F32, tag="st2", name=f"st2_{g}")
                for i in range(2):
                    nc.tensor.matmul(
                        st2[:, i * TQ : (i + 1) * TQ],
                        lhsT=kt[i * HD : (i + 1) * HD, ks],
                        rhs=qt[i * HD : (i + 1) * HD, qs],
                        start=True,
                        stop=True,
                    )
                st2_live[g] = st2

            def exp_stage(g):
                st2 = st2_live.pop(g)
                ex = expool.tile([P, 2 * TQ], F16, tag="ex", name=f"ex_{g}")
                nc.scalar.activation(
                    out=ex,
                    in_=st2,
                    func=mybir.ActivationFunctionType.Exp,
                    scale=1.0 / np.sqrt(HD),
                )
                ex_live[g] = ex

            def av_stage(g):
                qb, r = divmod(g, 32)
                p, kb = divmod(r, 16)
                if p == 0 and kb == 0:
                    oevp = [
                        tmps.tile(
                            [P, TQ], F32, tag=f"oevp{pp}",
                            name=f"oevp{pp}_{qb}", bufs=2,
                        )
                        for pp in range(2)
                    ]
                    den4 = tmps.tile(
                        [P, TQ], F32, tag="den4", name=f"den4_{qb}", bufs=2
                    )
                    nc.vector.memset(den4, 1.0)
                    qdat[qb] = (oevp, den4)
                if kb == 0:
                    oau_live[(qb, p)] = [
                        psO.tile([VW, TQ], F32, tag="oau", name=f"oau{i}_{qb}{p}")
                        for i in range(2)
                    ]
                oau = oau_live[(qb, p)]
                ex = ex_live.pop(g)
                for i in range(2):
                    h = 2 * p + i
                    nc.tensor.matmul(
                        oau[i],
                        lhsT=va_list[kb][:, h * VW : h * VW + VW],
                        rhs=ex[:, i * TQ : (i + 1) * TQ],
                        start=(kb == 0),
                        stop=(kb == NKB - 1),
                    )
                if kb == NKB - 1:
                    oevp, den4 = qdat[qb]
                    for i in range(2):
                        nc.vector.tensor_copy(
                            oevp[p][i * HD : (i + 1) * HD, :], oau[i][0:HD, :]
                        )
                        r0 = 32 * (2 * p + i)
                        nc.vector.tensor_copy(
                            den4[r0 : r0 + 1, :], oau[i][HD : HD + 1, :]
                        )
                    del oau_live[(qb, p)]
                    if p == 0:
                        add_filler(qb * 32 + 22, 900.0, lambda q=qb: fin_pair(q, 0))
                    else:
                        add_filler(qb * 32 + 36, 900.0, lambda q=qb: fin_pair(q, 1))
                        for tch in range(TQ // P):
                            add_filler(
                                qb * 32 + 38 + 4 * tch,
                                1200.0,
                                lambda q=qb, t=tch: yproj_tch(q, t),
                            )

            # ---- finalize: per-pair reciprocal chain, then y projection ----
            def fin_pair(qb, p):
                oevp, den4 = qdat[qb]
                o_sb = tmps.tile(
                    [P, TQ], F16, tag=f"osb{p}", name=f"osb{p}_{qb}", bufs=2
                )
                odat.setdefault(qb, [None, None])[p] = o_sb
                denT = psW.tile([P, 2 * HPC], F32, tag="aux", name=f"denT{qb}{p}")
                for c in range(4):
                    nc.tensor.matmul(
                        denT[:, c * 2 : (c + 1) * 2],
                        lhsT=den4[:, c * P : (c + 1) * P],
                        rhs=esel_t[:, 2 * p : 2 * p + 2],
                        start=True,
                        stop=True,
                    )
                rdenT = tmps.tile([P, 2 * HPC], F32, tag="rdenT")
                nc.vector.reciprocal(rdenT, denT)
                rden_ps = psW.tile([2, TQ], F32, tag="aux", name=f"rdps{qb}{p}")
                for c in range(4):
                    nc.tensor.transpose(
                        rden_ps[:, c * P : (c + 1) * P],
                        rdenT[:, c * 2 : (c + 1) * 2],
                        ident_t,
                    )
                rden2 = tmps.tile([2, TQ], F32R, tag="rden2")
                with nc.allow_low_precision(reason="f32r round of 1/den"):
                    nc.vector.tensor_copy(rden2, rden_ps)
                bc = psW.tile([P, TQ], F32, tag="aux", name=f"bc{qb}{p}")
                nc.tensor.matmul(bc, lhsT=emat_t, rhs=rden2, start=True, stop=True)
                nc.vector.tensor_mul(o_sb, oevp[p], bc)
                if p == 1:
                    del qdat[qb]

            def yproj_tch(qb, tch):
                # full C row-block per unit: N=1024 matmuls into a scores-
                # pool slot, f16 staging, one DMA.
                o_sb = odat[qb]
                yp = psS.tile([P, 2 * TQ], F32, tag="st2", name=f"yp{qb}{tch}")
                for cch in range(2):  # matmul out must stay within one PSUM bank
                    for kb in range(2):
                        nc.tensor.matmul(
                            yp[:, cch * TQ : (cch + 1) * TQ],
                            lhsT=o_sb[kb][:, tch * P : (tch + 1) * P],
                            rhs=wp_big[:, kb * C + cch * TQ : kb * C + (cch + 1) * TQ],
                            start=(kb == 0),
                            stop=(kb == 1),
                        )
                ysb = tmps.tile([P, 2 * TQ], F32, tag="ysb")
                nc.vector.tensor_copy(ysb, yp)
                r0 = qb * TQ + tch * P
                nc.sync.dma_start(out=y_d[r0 : r0 + P, :], in_=ysb)
                if tch == TQ // P - 1:
                    del odat[qb]

            # ---- prologue: k(pair0, n0) and q(pair0, n0) directly ----
            proj_qk_tile(2, 0)
            proj_qk_tile(0, 0)

            # ---- seed filler units with deadlines (j-space) ----
            # v(tb): consumed by AV(qb0, p0, kb=tb) at j = tb+2.
            for tb in range(NKB):
                add_filler(max(tb, 1), 1250.0, lambda t=tb: proj_v_tb(t))
            # k tiles: pair0 scores(kb=4nb) at j=4nb; pair1 at j=16+4nb.
            for nb in range(1, NQB):
                add_filler(4 * nb - 1, 2100.0, lambda n=nb: proj_qk_tile(2, n))
            add_filler(13, 2100.0, lambda: proj_qk_tile(3, 0))
            for nb in range(1, NQB):
                add_filler(16 + 4 * nb - 1, 2100.0, lambda n=nb: proj_qk_tile(3, n))
            # q tiles: qb=nb pair0 at j=32nb; pair1 at j=32nb+16.
            add_filler(14, 2100.0, lambda: proj_qk_tile(1, 0))
            for nb in range(1, NQB):
                add_filler(32 * nb - 3, 2100.0, lambda n=nb: proj_qk_tile(0, n))
                add_filler(32 * nb + 13, 2100.0, lambda n=nb: proj_qk_tile(1, n))

            # ---- main pipelined loop ----
            for j in range(NT + 2):
                pump(j)
                if j < NT:
                    scores_stage(j)
                if 1 <= j <= NT:
                    exp_stage(j - 1)
                if 2 <= j:
                    av_stage(j - 2)
            while fillers:
                _, _, _, fn = heapq.heappop(fillers)
                fn()

    nc.compile()
    return nc


def _get_program():
    global _PROGRAM
    if _PROGRAM is None:
        _PROGRAM = _build_program()
    return _PROGRAM


def _eo(w):
    """[64, C] head rows -> [even0:16; odd0:16; even16:32; odd16:32] so the
    RoPE partner is at p^16 within a 32-partition block (stream_shuffle)."""
    e = w[0::2]
    o = w[1::2]
    return np.concatenate([e[0:16], o[0:16], e[16:32], o[16:32]], axis=0)


def _host_prep(x, cos, sin, w_qkv, w_proj):
    """Build the 8 per-core input maps."""
    f16 = np.float16
    xT = [np.ascontiguousarray(x[b].T).astype(f16) for b in range(B)]  # [C, T]

    cosT = np.ascontiguousarray(cos.T)  # [32, T]
    sinT = np.ascontiguousarray(sin.T)
    # per-head 64-row block matching _eo: rows [e0:16, o0:16, e16:32, o16:32]
    cc64 = np.concatenate([cosT[0:16], cosT[0:16], cosT[16:32], cosT[16:32]])
    ss64 = np.concatenate([-sinT[0:16], sinT[0:16], -sinT[16:32], sinT[16:32]])
    cc = np.tile(cc64, (2, 1)).astype(f16)  # [128, T]
    ss = np.tile(ss64, (2, 1)).astype(f16)
    emat = np.zeros((2, P), dtype=np.float32)
    for i in range(2):
        emat[i, i * HD : (i + 1) * HD] = 1.0
    ident = np.eye(P, dtype=np.float32)
    esel = np.zeros((P, HPC), dtype=np.float32)
    for j in range(HPC):
        esel[32 * j, j] = 1.0

    wq = w_qkv[0:C]
    wk = w_qkv[C : 2 * C]
    wv = w_qkv[2 * C : 3 * C]

    in_maps = []
    for core in range(N_CORES):
        b = core // 4
        h0 = 4 * (core % 4)
        heads = [h0, h0 + 1, h0 + 2, h0 + 3]
        blocks = []
        for pair in range(2):
            ha, hb = heads[2 * pair], heads[2 * pair + 1]
            blocks.append(
                np.concatenate(
                    [_eo(wq[ha * HD : ha * HD + HD]),
                     _eo(wq[hb * HD : hb * HD + HD])],
                    axis=0,
                )
            )
        for pair in range(2):
            ha, hb = heads[2 * pair], heads[2 * pair + 1]
            blocks.append(
                np.concatenate(
                    [_eo(wk[ha * HD : ha * HD + HD]),
                     _eo(wk[hb * HD : hb * HD + HD])],
                    axis=0,
                )
            )
        wqkT = np.ascontiguousarray(
            np.concatenate(blocks, axis=0).T
        ).astype(f16)  # [C, 512]
        wvT = np.ascontiguousarray(
            wv[h0 * HD : h0 * HD + GC].T
        ).astype(f16)  # [C, 256]
        wpT = np.ascontiguousarray(
            w_proj[:, h0 * HD : h0 * HD + GC].T
        ).astype(f16)  # [256, C]
        in_maps.append(
            {
                "xT": xT[b],
                "wqkT": wqkT,
                "wvT": wvT,
                "wpT": wpT,
                "cc": cc,
                "ss": ss,
                "emat": emat,
                "ident": ident,
                "esel": esel,
            }
        )
    return in_maps


def kernel(x, cos, sin, mask, w_qkv, w_proj, _trace=False, _tmpdir=None):
    x = np.asarray(x, dtype=np.float32)
    cos = np.asarray(cos, dtype=np.float32)
    sin = np.asarray(sin, dtype=np.float32)
    w_qkv = np.asarray(w_qkv, dtype=np.float32)
    w_proj = np.asarray(w_proj, dtype=np.float32)
    # mask is all-ones in this problem spec: no-op in the math.

    nc = _get_program()
    in_maps = _host_prep(x, cos, sin, w_qkv, w_proj)
    res = run_bass_kernel_spmd(
        nc, in_maps, list(range(N_CORES)), trace=_trace, tmpdir=_tmpdir
    )
    out = np.empty((B, T, C), dtype=np.float32)
    for b in range(B):
        acc = res.results[4 * b]["y"].astype(np.float32).copy()
        for g in range(1, 4):
            acc += res.results[4 * b + g]["y"]
        out[b] = acc
    kernel._last_exec_time_ns = res.exec_time_ns
    return out
